# revision 10
# baseline (speedup 1.0000x reference)
"""Trainium2 Bass kernel for nn_AxialAttention (8 NeuronCores, SPMD data-parallel).

Reference computation (per flattened batch row of B = b*h*d2 = 512):
    S = Q @ K.T / sqrt(64)                (512 x 512)
    S = where(attention_mask == 0, -1e9, S)
    P0 = softmax(S, axis=-1)
    probs = P0 * head_mask                (second output)
    out = probs @ V                       (first output, reshaped)

Device strategy (per core, 64 batch rows):
    Everything is computed in "transposed" layout (k on partitions, q on the
    free dim) so both matmuls contract on the partition axis with zero
    on-device transposes:
      mm1   : S_T = K @ Q.T       row-packed: 2 batches share the PE array
                                  (K=64 each, row groups 0/64)
      exp   : E = exp(0.125*S_T)  ScalarE, psum -> sbuf bf16, half-batch tiles
      Esum  : sum E over the 4 k-chunks on GpSimd (2 bf16 adds)
      mmden : denom row i of a group psum tile via shifted one-hot lhsT
      recip : grouped approx reciprocal on VectorE
      bcast : inv broadcast to 128 partitions via a DRAM bounce
      t1    : E * hm.T            VectorE bf16 2x
      probsT: t1 * inv_bc         VectorE bf16 2x  -> DMA out (bf16)
      mm2   : out_T = V.T @ probsT -> (64, 512) -> ScalarE copy -> DMA
The host reassembles/transposes the outputs and casts bf16 -> f32.
"""

import sys

if "/opt/trn_rl_repo" not in sys.path:
    sys.path.insert(0, "/opt/trn_rl_repo")

import numpy as np
import ml_dtypes

BF16 = ml_dtypes.bfloat16

B_FULL = 512      # b*h*d2 = 2*8*32
L = 512           # d1 (attention/sequence length)
D = 64            # head dim
N_CORES = 8
BPC = B_FULL // N_CORES   # 64 batch rows per core
KC = L // 128             # 4 chunks of the k axis
GROUP = 8                 # batches per reciprocal group
SCALE = 0.125             # 1/sqrt(64)
MASK_NEG = -8000.0        # additive mask; exp(0.125*-8000) == 0 in f32

_compiled = {}  # masked(bool) -> compiled Bacc


def build_nc(masked: bool):
    """Build + compile the per-core Bass program (shared by all 8 cores)."""
    import concourse.bacc as bacc
    import concourse.mybir as mybir
    import concourse.tile as tile

    dt = mybir.dt
    f32 = dt.float32
    bf16 = dt.bfloat16
    Alu = mybir.AluOpType
    Act = mybir.ActivationFunctionType

    nc = bacc.Bacc("TRN2", target_bir_lowering=False, debug=False,
                   num_devices=N_CORES)

    qT = nc.declare_dram_parameter("qT", [BPC, D, L], bf16, isOutput=False)
    kT = nc.declare_dram_parameter("kT", [BPC, D, L], bf16, isOutput=False)
    v = nc.declare_dram_parameter("v", [BPC, L, D], bf16, isOutput=False)
    hmT = nc.declare_dram_parameter("hmT", [L, L], bf16, isOutput=False)
    if masked:
        maskT = nc.declare_dram_parameter("maskT", [L, L], bf16, isOutput=False)
    probsT = nc.declare_dram_parameter("probsT", [BPC, L, L], bf16, isOutput=True)
    outT = nc.declare_dram_parameter("outT", [BPC, D, L], bf16, isOutput=True)

    NG = BPC // GROUP
    PAIRS = GROUP // 2

    with tile.TileContext(nc) as tc:
        with (
            tc.tile_pool(name="const", bufs=1) as constp,
            tc.tile_pool(name="qk", bufs=2) as qkp,
            tc.tile_pool(name="vp", bufs=2) as vp,
            tc.tile_pool(name="den", bufs=2) as denp,
            tc.tile_pool(name="work", bufs=4) as workp,
            tc.tile_pool(name="esum", bufs=3) as esump,
            tc.tile_pool(name="t1p", bufs=GROUP + 2) as t1p,
            tc.tile_pool(name="bc", bufs=2) as bcp,
            tc.tile_pool(name="outp", bufs=2) as outp,
            tc.tile_pool(name="psS", bufs=2, space="PSUM") as psS,
            tc.tile_pool(name="psden", bufs=2, space="PSUM") as psden,
            tc.tile_pool(name="psO", bufs=2, space="PSUM") as psO,
            tc.tile_pool(name="dram", bufs=2, space="DRAM") as dramp,
        ):
            # --- persistent constants
            hmT_sb = constp.tile([128, KC, L], bf16, tag="hmT")
            nc.sync.dma_start(
                hmT_sb[:], hmT.ap().rearrange("(kc p) q -> p kc q", p=128))
            if masked:
                maskT_sb = constp.tile([128, KC, L], bf16, tag="maskT")
                nc.sync.dma_start(
                    maskT_sb[:], maskT.ap().rearrange("(kc p) q -> p kc q", p=128))
            # onehot_pad[:, GROUP] == 1, else 0.  Sliced as
            # onehot_pad[:, GROUP-i : 2*GROUP-i] it is a (128, GROUP) lhsT
            # whose only nonzero column is column i — the denominator matmul
            # then lands batch i's row sums on psum partition i.
            onehot_pad = constp.tile([128, 2 * GROUP + 1], bf16, tag="onehot")
            nc.vector.memset(onehot_pad[:], 0.0)
            nc.vector.memset(onehot_pad[:, GROUP:GROUP + 1], 1.0)

            for g in range(NG):
                b0 = g * GROUP
                # --- group input DMAs.  q/k are loaded pair-packed: batch
                # 2j on partitions 0..63, batch 2j+1 on partitions 64..127,
                # so two batches' S_T matmuls share the PE array (row tiling).
                qT_g = qkp.tile([128, PAIRS, L], bf16, tag="qT")
                kT_g = qkp.tile([128, PAIRS, L], bf16, tag="kT")
                v_g = vp.tile([128, GROUP, KC, D], bf16, tag="v")
                nc.sync.dma_start(
                    qT_g[:],
                    qT.ap()[b0:b0 + GROUP].rearrange(
                        "(j two) d l -> (two d) j l", two=2))
                nc.sync.dma_start(
                    kT_g[:],
                    kT.ap()[b0:b0 + GROUP].rearrange(
                        "(j two) d l -> (two d) j l", two=2))
                nc.sync.dma_start(
                    v_g[:],
                    v.ap()[b0:b0 + GROUP].rearrange(
                        "b (kc p) d -> p b kc d", p=128))

                denrow = denp.tile([GROUP, L], f32, tag="den")
                invrow = denp.tile([GROUP, L], f32, tag="inv")
                invrow_bf = denp.tile([GROUP, L], bf16, tag="invbf")
                den_ps = psden.tile([GROUP, L], f32, tag="dps")

                E_tiles = []
                for j in range(PAIRS):
                    E_a = workp.tile([128, KC, L], bf16, tag="E")
                    E_b = workp.tile([128, KC, L], bf16, tag="E")
                    for h in range(2):
                        S_a = psS.tile([128, 2, L], f32, tag="S")
                        S_b = psS.tile([128, 2, L], f32, tag="S")
                        for c in range(2):
                            kc = 2 * h + c
                            ksl = slice(kc * 128, (kc + 1) * 128)
                            nc.tensor.matmul(
                                S_a[:, c, :], kT_g[0:64, j, ksl],
                                qT_g[0:64, j, :],
                                start=True, stop=True, tile_position=(0, 0))
                            nc.tensor.matmul(
                                S_b[:, c, :], kT_g[64:128, j, ksl],
                                qT_g[64:128, j, :],
                                start=True, stop=True, tile_position=(64, 0))
                        if masked:
                            nc.vector.tensor_tensor(
                                out=S_a[:], in0=S_a[:],
                                in1=maskT_sb[:, 2 * h:2 * h + 2, :], op=Alu.add)
                            nc.vector.tensor_tensor(
                                out=S_b[:], in0=S_b[:],
                                in1=maskT_sb[:, 2 * h:2 * h + 2, :], op=Alu.add)
                        nc.scalar.activation(
                            E_a[:, 2 * h:2 * h + 2, :], S_a[:], Act.Exp,
                            scale=SCALE)
                        nc.scalar.activation(
                            E_b[:, 2 * h:2 * h + 2, :], S_b[:], Act.Exp,
                            scale=SCALE)
                    E_tiles.append(E_a)
                    E_tiles.append(E_b)

                t1_tiles = []
                for i in range(GROUP):
                    E_sb = E_tiles[i]
                    # k-chunk pre-sum via accumulating DMA (software DGE);
                    # costs no engine compute, then one denominator matmul
                    # instead of four.
                    esum = esump.tile([128, L], bf16, tag="esum")
                    nc.sync.dma_start(esum[:], E_sb[:, 0, :])
                    for cc in range(1, KC):
                        nc.gpsimd.dma_start(esum[:], E_sb[:, cc, :],
                                            accum_op=Alu.add)
                    oh = onehot_pad[:, GROUP - i:2 * GROUP - i]
                    nc.tensor.matmul(
                        den_ps[:], oh, esum[:],
                        start=(i == 0), stop=(i == GROUP - 1),
                        skip_group_check=True)
                    # t1 = E * hm.T
                    t1 = t1p.tile([128, KC, L], bf16, tag="t1")
                    nc.vector.tensor_tensor(
                        out=t1[:], in0=E_sb[:], in1=hmT_sb[:], op=Alu.mult)
                    t1_tiles.append(t1)

                # --- grouped reciprocal of the softmax denominators
                nc.vector.tensor_copy(denrow[:], den_ps[:])
                nc.vector.reciprocal_approx_fast(invrow[:], denrow[:])
                nc.vector.tensor_copy(invrow_bf[:], invrow[:])
                # broadcast inv across partitions via a DRAM bounce
                inv_dram = dramp.tile([GROUP, L], bf16, tag="invd")
                nc.sync.dma_start(inv_dram[:], invrow_bf[:])
                invbc_g = bcp.tile([128, GROUP, L], bf16, tag="invbc")
                nc.sync.dma_start(
                    invbc_g[:],
                    inv_dram[None, :, :].to_broadcast((128, GROUP, L)))

                out_g = outp.tile([128, PAIRS, L], bf16, tag="out")
                for j in range(PAIRS):
                    probs_pair = []
                    for half in range(2):
                        i = 2 * j + half
                        b = b0 + i
                        probs_sb = workp.tile([128, KC, L], bf16, tag="probs")
                        nc.vector.tensor_tensor(
                            out=probs_sb[:],
                            in0=t1_tiles[i][:],
                            in1=invbc_g[:, i, None, :].to_broadcast(
                                (128, KC, L)),
                            op=Alu.mult)
                        nc.sync.dma_start(
                            probsT.ap()[b].rearrange(
                                "(kc p) q -> p kc q", p=128),
                            probs_sb[:])
                        probs_pair.append(probs_sb)
                    # mm2, column-packed: batch 2j -> psum partitions 0..63,
                    # batch 2j+1 -> partitions 64..127, same bank.
                    O_ps = psO.tile([128, L], f32, tag="O")
                    for kc in range(KC):
                        nc.tensor.matmul(
                            O_ps[0:D, :], v_g[:, 2 * j, kc, :],
                            probs_pair[0][:, kc, :],
                            start=(kc == 0), stop=(kc == KC - 1),
                            tile_position=(0, 0), skip_group_check=True)
                        nc.tensor.matmul(
                            O_ps[D:128, :], v_g[:, 2 * j + 1, kc, :],
                            probs_pair[1][:, kc, :],
                            start=(kc == 0), stop=(kc == KC - 1),
                            tile_position=(0, 64), skip_group_check=True)
                    nc.scalar.copy(out_g[:, j, :], O_ps[:])

                nc.sync.dma_start(
                    outT.ap()[b0:b0 + GROUP].rearrange(
                        "(j two) d l -> (two d) j l", two=2),
                    out_g[:])

    nc.compile()
    return nc


def _get_nc(masked: bool):
    if masked not in _compiled:
        _compiled[masked] = build_nc(masked)
    return _compiled[masked]


def _bf16_to_f32(x):
    # fast bf16 -> f32: widen the raw bits
    u = np.asarray(x).view(np.uint16).astype(np.uint32) << 16
    return u.view(np.float32)


def prepare_inputs(q, k, v, attention_mask, head_mask):
    """Host-side shard + layout prep. Returns (in_maps, masked)."""
    q = np.asarray(q, dtype=np.float32)
    k = np.asarray(k, dtype=np.float32)
    v = np.asarray(v, dtype=np.float32)
    am = np.asarray(attention_mask, dtype=np.float32)
    hm = np.asarray(head_mask, dtype=np.float32)

    # flatten to (B, L, D) exactly like the reference (axial dim -> -2)
    qf = np.moveaxis(q, 2, -2).reshape(B_FULL, L, D)
    kf = np.moveaxis(k, 2, -2).reshape(B_FULL, L, D)
    vf = np.moveaxis(v, 2, -2).reshape(B_FULL, L, D)

    qT = np.ascontiguousarray(qf.transpose(0, 2, 1)).astype(BF16)   # (B, D, L)
    kT = np.ascontiguousarray(kf.transpose(0, 2, 1)).astype(BF16)   # (B, D, L)
    vb = np.ascontiguousarray(vf).astype(BF16)                      # (B, L, D)
    hmT = np.ascontiguousarray(hm[0].T).astype(BF16)                # (Lk, Lq)

    masked = bool((am == 0).any())
    maskT = None
    if masked:
        maskT = np.ascontiguousarray(
            np.where(am[0] == 0, np.float32(MASK_NEG), np.float32(0.0)).T
        ).astype(BF16)

    in_maps = []
    for c in range(N_CORES):
        s = slice(c * BPC, (c + 1) * BPC)
        m = {"qT": qT[s], "kT": kT[s], "v": vb[s], "hmT": hmT}
        if masked:
            m["maskT"] = maskT
        in_maps.append(m)
    return in_maps, masked


def assemble_outputs(results):
    """Gather per-core results -> full (out, probs) f32 arrays."""
    probs = np.empty((B_FULL, L, L), dtype=np.float32)
    outf = np.empty((B_FULL, L, D), dtype=np.float32)
    for c in range(N_CORES):
        s = slice(c * BPC, (c + 1) * BPC)
        pT = _bf16_to_f32(results[c]["probsT"])     # (BPC, Lk, Lq)
        probs[s] = pT.swapaxes(1, 2)                # -> (BPC, Lq, Lk)
        oT = _bf16_to_f32(results[c]["outT"])       # (BPC, D, L)
        outf[s] = oT.swapaxes(1, 2)                 # -> (BPC, L, D)
    out5 = outf.reshape(2, 8, 32, L, D)             # (b, h, d2, d1, dim)
    out = np.ascontiguousarray(np.moveaxis(out5, 3, 2))  # (b, h, d1, d2, dim)
    return out, probs


def run_on_cores(in_maps, masked, **kwargs):
    from concourse.bass_utils import run_bass_kernel_spmd
    nc = _get_nc(masked)
    return run_bass_kernel_spmd(nc, in_maps, core_ids=list(range(N_CORES)),
                                **kwargs)


def kernel(q, k, v, attention_mask, head_mask):
    in_maps, masked = prepare_inputs(q, k, v, attention_mask, head_mask)
    res = run_on_cores(in_maps, masked)
    return assemble_outputs(res.results)


# revision 12
# speedup vs baseline: 1.8723x; 1.8723x over previous
"""Trainium2 Bass kernel for nn_AxialAttention (8 NeuronCores, SPMD data-parallel).

Reference computation (per flattened batch row of B = b*h*d2 = 512):
    S = Q @ K.T / sqrt(64)                (512 x 512)
    S = where(attention_mask == 0, -1e9, S)
    P0 = softmax(S, axis=-1)
    probs = P0 * head_mask                (second output)
    out = probs @ V                       (first output, reshaped)

Device strategy (per core, 64 batch rows):
    Everything is computed in "transposed" layout (k on partitions, q on the
    free dim) so both matmuls contract on the partition axis with zero
    on-device transposes:
      mm1   : S_T = K @ Q.T       row-packed: 2 batches share the PE array
                                  (K=64 each, row groups 0/64)
      exp   : E = exp(0.125*S_T)  ScalarE, psum -> sbuf bf16, half-batch tiles
      Esum  : sum E over the 4 k-chunks on GpSimd (2 bf16 adds)
      mmden : denom row i of a group psum tile via shifted one-hot lhsT
      recip : grouped approx reciprocal on VectorE
      bcast : inv broadcast to 128 partitions via a DRAM bounce
      t1    : E * hm.T            VectorE bf16 2x
      probsT: t1 * inv_bc         VectorE bf16 2x  -> DMA out (bf16)
      mm2   : out_T = V.T @ probsT -> (64, 512) -> ScalarE copy -> DMA
The host reassembles/transposes the outputs and casts bf16 -> f32.
"""

import sys

if "/opt/trn_rl_repo" not in sys.path:
    sys.path.insert(0, "/opt/trn_rl_repo")

import numpy as np
import ml_dtypes

BF16 = ml_dtypes.bfloat16

B_FULL = 512      # b*h*d2 = 2*8*32
L = 512           # d1 (attention/sequence length)
D = 64            # head dim
N_CORES = 8
BPC = B_FULL // N_CORES   # 64 batch rows per core
KC = L // 128             # 4 chunks of the k axis
GROUP = 8                 # batches per reciprocal group
SCALE = 0.125             # 1/sqrt(64)
MASK_NEG = -8000.0        # additive mask; exp(0.125*-8000) == 0 in f32

_compiled = {}  # masked(bool) -> compiled Bacc


def build_nc(masked: bool):
    """Build + compile the per-core Bass program (shared by all 8 cores)."""
    import concourse.bacc as bacc
    import concourse.mybir as mybir
    import concourse.tile as tile

    dt = mybir.dt
    f32 = dt.float32
    bf16 = dt.bfloat16
    Alu = mybir.AluOpType
    Act = mybir.ActivationFunctionType

    nc = bacc.Bacc("TRN2", target_bir_lowering=False, debug=False,
                   num_devices=N_CORES)

    qT = nc.declare_dram_parameter("qT", [BPC, D, L], bf16, isOutput=False)
    kT = nc.declare_dram_parameter("kT", [BPC, D, L], bf16, isOutput=False)
    v = nc.declare_dram_parameter("v", [BPC, L, D], bf16, isOutput=False)
    hmT = nc.declare_dram_parameter("hmT", [L, L], bf16, isOutput=False)
    if masked:
        maskT = nc.declare_dram_parameter("maskT", [L, L], bf16, isOutput=False)
    probsT = nc.declare_dram_parameter("probsT", [BPC, L, L], bf16, isOutput=True)
    outT = nc.declare_dram_parameter("outT", [BPC, D, L], bf16, isOutput=True)

    NG = BPC // GROUP
    PAIRS = GROUP // 2

    with tile.TileContext(nc) as tc:
        with (
            tc.tile_pool(name="const", bufs=1) as constp,
            tc.tile_pool(name="qk", bufs=2) as qkp,
            tc.tile_pool(name="vp", bufs=2) as vp,
            tc.tile_pool(name="den", bufs=2) as denp,
            tc.tile_pool(name="work", bufs=4) as workp,
            tc.tile_pool(name="esum", bufs=3) as esump,
            tc.tile_pool(name="t1p", bufs=GROUP + 2) as t1p,
            tc.tile_pool(name="bc", bufs=2) as bcp,
            tc.tile_pool(name="outp", bufs=2) as outp,
            tc.tile_pool(name="psS", bufs=2, space="PSUM") as psS,
            tc.tile_pool(name="psden", bufs=2, space="PSUM") as psden,
            tc.tile_pool(name="psO", bufs=2, space="PSUM") as psO,
            tc.tile_pool(name="dram", bufs=2, space="DRAM") as dramp,
        ):
            # --- persistent constants
            hmT_sb = constp.tile([128, KC, L], bf16, tag="hmT")
            nc.sync.dma_start(
                hmT_sb[:], hmT.ap().rearrange("(kc p) q -> p kc q", p=128))
            if masked:
                maskT_sb = constp.tile([128, KC, L], bf16, tag="maskT")
                nc.sync.dma_start(
                    maskT_sb[:], maskT.ap().rearrange("(kc p) q -> p kc q", p=128))
            # onehot_pad[:, GROUP] == 1, else 0.  Sliced as
            # onehot_pad[:, GROUP-i : 2*GROUP-i] it is a (128, GROUP) lhsT
            # whose only nonzero column is column i — the denominator matmul
            # then lands batch i's row sums on psum partition i.
            onehot_pad = constp.tile([128, 2 * GROUP + 1], bf16, tag="onehot")
            nc.vector.memset(onehot_pad[:], 0.0)
            nc.vector.memset(onehot_pad[:, GROUP:GROUP + 1], 1.0)

            for g in range(NG):
                b0 = g * GROUP
                # --- group input DMAs.  q/k are loaded pair-packed: batch
                # 2j on partitions 0..63, batch 2j+1 on partitions 64..127,
                # so two batches' S_T matmuls share the PE array (row tiling).
                qT_g = qkp.tile([128, PAIRS, L], bf16, tag="qT")
                kT_g = qkp.tile([128, PAIRS, L], bf16, tag="kT")
                v_g = vp.tile([128, GROUP, KC, D], bf16, tag="v")
                nc.sync.dma_start(
                    qT_g[:],
                    qT.ap()[b0:b0 + GROUP].rearrange(
                        "(j two) d l -> (two d) j l", two=2))
                nc.sync.dma_start(
                    kT_g[:],
                    kT.ap()[b0:b0 + GROUP].rearrange(
                        "(j two) d l -> (two d) j l", two=2))
                nc.sync.dma_start(
                    v_g[:],
                    v.ap()[b0:b0 + GROUP].rearrange(
                        "b (kc p) d -> p b kc d", p=128))

                denrow = denp.tile([GROUP, L], f32, tag="den")
                dtmp0 = denp.tile([GROUP, L], f32, tag="dtmp0")
                dtmp1 = denp.tile([GROUP, L], f32, tag="dtmp1")
                invrow = denp.tile([GROUP, L], f32, tag="inv")
                invrow_bf = denp.tile([GROUP, L], bf16, tag="invbf")
                # one bank; col-group c rows 32c..32c+GROUP hold the partial
                # denominators over k-chunk c (batches on one-hot rows)
                den_ps = psden.tile([128, L], f32, tag="dps")

                E_tiles = []
                for j in range(PAIRS):
                    E_a = workp.tile([128, KC, L], bf16, tag="E")
                    E_b = workp.tile([128, KC, L], bf16, tag="E")
                    for h in range(2):
                        S_a = psS.tile([128, 2, L], f32, tag="S")
                        S_b = psS.tile([128, 2, L], f32, tag="S")
                        for c in range(2):
                            kc = 2 * h + c
                            ksl = slice(kc * 128, (kc + 1) * 128)
                            nc.tensor.matmul(
                                S_a[:, c, :], kT_g[0:64, j, ksl],
                                qT_g[0:64, j, :],
                                start=True, stop=True, tile_position=(0, 0))
                            nc.tensor.matmul(
                                S_b[:, c, :], kT_g[64:128, j, ksl],
                                qT_g[64:128, j, :],
                                start=True, stop=True, tile_position=(64, 0))
                        if masked:
                            nc.vector.tensor_tensor(
                                out=S_a[:], in0=S_a[:],
                                in1=maskT_sb[:, 2 * h:2 * h + 2, :], op=Alu.add)
                            nc.vector.tensor_tensor(
                                out=S_b[:], in0=S_b[:],
                                in1=maskT_sb[:, 2 * h:2 * h + 2, :], op=Alu.add)
                        nc.scalar.activation(
                            E_a[:, 2 * h:2 * h + 2, :], S_a[:], Act.Exp,
                            scale=SCALE)
                        nc.scalar.activation(
                            E_b[:, 2 * h:2 * h + 2, :], S_b[:], Act.Exp,
                            scale=SCALE)
                    E_tiles.append(E_a)
                    E_tiles.append(E_b)

                t1_tiles = []
                for i in range(GROUP):
                    E_sb = E_tiles[i]
                    # denominator: 4 col-packed one-hot matmuls run
                    # concurrently in the PE array's 4 column groups,
                    # landing chunk c's row sums on psum partition 32c+i.
                    oh = onehot_pad[:, GROUP - i:2 * GROUP - i]
                    for kc in range(KC):
                        nc.tensor.matmul(
                            den_ps[32 * kc:32 * kc + GROUP, :], oh,
                            E_sb[:, kc, :],
                            start=(i == 0), stop=(i == GROUP - 1),
                            tile_position=(0, 32 * kc),
                            skip_group_check=True)
                    # t1 = E * hm.T
                    t1 = t1p.tile([128, KC, L], bf16, tag="t1")
                    nc.vector.tensor_tensor(
                        out=t1[:], in0=E_sb[:], in1=hmT_sb[:], op=Alu.mult)
                    t1_tiles.append(t1)

                # --- fold the 4 chunk blocks, then grouped reciprocal
                nc.vector.tensor_copy(dtmp0[:], den_ps[0:GROUP, :])
                nc.vector.tensor_tensor(
                    out=dtmp1[:], in0=dtmp0[:], in1=den_ps[32:32 + GROUP, :],
                    op=Alu.add)
                nc.vector.tensor_tensor(
                    out=dtmp0[:], in0=dtmp1[:], in1=den_ps[64:64 + GROUP, :],
                    op=Alu.add)
                nc.vector.tensor_tensor(
                    out=denrow[:], in0=dtmp0[:], in1=den_ps[96:96 + GROUP, :],
                    op=Alu.add)
                nc.vector.reciprocal_approx_fast(invrow[:], denrow[:])
                nc.vector.tensor_copy(invrow_bf[:], invrow[:])
                # broadcast inv across partitions via a DRAM bounce
                inv_dram = dramp.tile([GROUP, L], bf16, tag="invd")
                nc.sync.dma_start(inv_dram[:], invrow_bf[:])
                invbc_g = bcp.tile([128, GROUP, L], bf16, tag="invbc")
                nc.sync.dma_start(
                    invbc_g[:],
                    inv_dram[None, :, :].to_broadcast((128, GROUP, L)))

                out_g = outp.tile([128, PAIRS, L], bf16, tag="out")
                for j in range(PAIRS):
                    probs_pair = []
                    for half in range(2):
                        i = 2 * j + half
                        b = b0 + i
                        probs_sb = workp.tile([128, KC, L], bf16, tag="probs")
                        nc.vector.tensor_tensor(
                            out=probs_sb[:],
                            in0=t1_tiles[i][:],
                            in1=invbc_g[:, i, None, :].to_broadcast(
                                (128, KC, L)),
                            op=Alu.mult)
                        nc.sync.dma_start(
                            probsT.ap()[b].rearrange(
                                "(kc p) q -> p kc q", p=128),
                            probs_sb[:])
                        probs_pair.append(probs_sb)
                    # mm2, column-packed: batch 2j -> psum partitions 0..63,
                    # batch 2j+1 -> partitions 64..127, same bank.
                    O_ps = psO.tile([128, L], f32, tag="O")
                    for kc in range(KC):
                        nc.tensor.matmul(
                            O_ps[0:D, :], v_g[:, 2 * j, kc, :],
                            probs_pair[0][:, kc, :],
                            start=(kc == 0), stop=(kc == KC - 1),
                            tile_position=(0, 0), skip_group_check=True)
                        nc.tensor.matmul(
                            O_ps[D:128, :], v_g[:, 2 * j + 1, kc, :],
                            probs_pair[1][:, kc, :],
                            start=(kc == 0), stop=(kc == KC - 1),
                            tile_position=(0, 64), skip_group_check=True)
                    nc.scalar.copy(out_g[:, j, :], O_ps[:])

                nc.sync.dma_start(
                    outT.ap()[b0:b0 + GROUP].rearrange(
                        "(j two) d l -> (two d) j l", two=2),
                    out_g[:])

    nc.compile()
    return nc


def _get_nc(masked: bool):
    if masked not in _compiled:
        _compiled[masked] = build_nc(masked)
    return _compiled[masked]


def _bf16_to_f32(x):
    # fast bf16 -> f32: widen the raw bits
    u = np.asarray(x).view(np.uint16).astype(np.uint32) << 16
    return u.view(np.float32)


def prepare_inputs(q, k, v, attention_mask, head_mask):
    """Host-side shard + layout prep. Returns (in_maps, masked)."""
    q = np.asarray(q, dtype=np.float32)
    k = np.asarray(k, dtype=np.float32)
    v = np.asarray(v, dtype=np.float32)
    am = np.asarray(attention_mask, dtype=np.float32)
    hm = np.asarray(head_mask, dtype=np.float32)

    # flatten to (B, L, D) exactly like the reference (axial dim -> -2)
    qf = np.moveaxis(q, 2, -2).reshape(B_FULL, L, D)
    kf = np.moveaxis(k, 2, -2).reshape(B_FULL, L, D)
    vf = np.moveaxis(v, 2, -2).reshape(B_FULL, L, D)

    qT = np.ascontiguousarray(qf.transpose(0, 2, 1)).astype(BF16)   # (B, D, L)
    kT = np.ascontiguousarray(kf.transpose(0, 2, 1)).astype(BF16)   # (B, D, L)
    vb = np.ascontiguousarray(vf).astype(BF16)                      # (B, L, D)
    hmT = np.ascontiguousarray(hm[0].T).astype(BF16)                # (Lk, Lq)

    masked = bool((am == 0).any())
    maskT = None
    if masked:
        maskT = np.ascontiguousarray(
            np.where(am[0] == 0, np.float32(MASK_NEG), np.float32(0.0)).T
        ).astype(BF16)

    in_maps = []
    for c in range(N_CORES):
        s = slice(c * BPC, (c + 1) * BPC)
        m = {"qT": qT[s], "kT": kT[s], "v": vb[s], "hmT": hmT}
        if masked:
            m["maskT"] = maskT
        in_maps.append(m)
    return in_maps, masked


def assemble_outputs(results):
    """Gather per-core results -> full (out, probs) f32 arrays."""
    probs = np.empty((B_FULL, L, L), dtype=np.float32)
    outf = np.empty((B_FULL, L, D), dtype=np.float32)
    for c in range(N_CORES):
        s = slice(c * BPC, (c + 1) * BPC)
        pT = _bf16_to_f32(results[c]["probsT"])     # (BPC, Lk, Lq)
        probs[s] = pT.swapaxes(1, 2)                # -> (BPC, Lq, Lk)
        oT = _bf16_to_f32(results[c]["outT"])       # (BPC, D, L)
        outf[s] = oT.swapaxes(1, 2)                 # -> (BPC, L, D)
    out5 = outf.reshape(2, 8, 32, L, D)             # (b, h, d2, d1, dim)
    out = np.ascontiguousarray(np.moveaxis(out5, 3, 2))  # (b, h, d1, d2, dim)
    return out, probs


def run_on_cores(in_maps, masked, **kwargs):
    from concourse.bass_utils import run_bass_kernel_spmd
    nc = _get_nc(masked)
    return run_bass_kernel_spmd(nc, in_maps, core_ids=list(range(N_CORES)),
                                **kwargs)


def kernel(q, k, v, attention_mask, head_mask):
    in_maps, masked = prepare_inputs(q, k, v, attention_mask, head_mask)
    res = run_on_cores(in_maps, masked)
    return assemble_outputs(res.results)
